# revision 17
# baseline (speedup 1.0000x reference)
"""Trainium2 kernel for the BaseSegHead NMS-detection problem.

Pipeline (full inputs in, full outputs out):
  pass A (device, data-parallel over masks): per-mask row/col occupancy
      stats of (seg_logits > 0) -> host turns them into boxes/non_empty.
  host glue (jax CPU, mirrors the reference math op-for-op): sigmoid
      scores, top-k candidates, class-aware greedy NMS, top max_segs.
  pass B (device, data-parallel over selected masks): bilinear 200->1024
      upsample as two chained PE matmuls per mask + >0 threshold to u8.
"""

import sys

sys.path.insert(0, "/opt/trn_rl_repo")

import numpy as np

import concourse.bass as bass
import concourse.mybir as mybir
from concourse.bass_utils import run_bass_kernel_spmd
from concourse.tile import TileContext

# problem constants (hardcoded per spec)
N, NCLS = 300, 81
C = NCLS - 1
FH = FW = 200
IH = IW = 1024
NMS_CAND = 1000
NMS_THR = 0.65
MAX_SEGS = 100
NCORES = 8

APM = 38   # masks per core, pass A (8*38 = 304 >= 300)
BPM = 13   # masks per core, pass B (8*13 = 104 >= 100)

F32 = mybir.dt.float32
BF16 = mybir.dt.bfloat16
U8 = mybir.dt.uint8


_SW_CTR = [0]


def _split_multi_waits(nc):
    """Move extra sem waits onto same-engine NoOps before each instruction.

    This walrus build accepts at most one sync wait per instruction
    (CTRL, S3_LW, ... encodings); Tile emits one wait per pending proc.
    """
    for f in nc.m.functions:
        for b in f.blocks:
            out = []
            changed = False
            for inst in b.instructions:
                si = inst.sync_info
                waits = list(si.on_wait) if si is not None else []
                if len(waits) > 1:
                    for w in waits[:-1]:
                        _SW_CTR[0] += 1
                        nop = mybir.InstNoOp(name=f"I-sw{_SW_CTR[0]}", ins=[], outs=[])
                        nop.engine = inst.engine
                        nop.sync_info = mybir.SyncInfo(on_wait=[w], on_update=[])
                        out.append(nop)
                    inst.sync_info = mybir.SyncInfo(
                        on_wait=[waits[-1]], on_update=list(si.on_update)
                    )
                    changed = True
                out.append(inst)
            if changed:
                b.instructions = out
                assert len(b.instructions) == len(out)


# ----------------------------------------------------------------- pass A

def _build_pass_a(split=True):
    nc = bass.Bass()
    segs = nc.dram_tensor("segs", [APM, FH, FW], F32, kind="ExternalInput")
    rowmax = nc.dram_tensor("rowmax", [FH, APM], BF16, kind="ExternalOutput")
    colsum = nc.dram_tensor("colsum", [1, APM * FW], F32, kind="ExternalOutput")

    W = APM * FW  # 7600
    segs_r = segs.rearrange("m r c -> r m c")

    with TileContext(nc) as tc:
        with (
            tc.tile_pool(name="const", bufs=1) as cpool,
            tc.tile_pool(name="sbuf", bufs=3) as pool,
            tc.tile_pool(name="psum", bufs=4, space="PSUM") as psum,
        ):
            ones = cpool.tile([128, 32], BF16, tag="ones")
            nc.vector.memset(ones[:], 1.0)
            rm0 = cpool.tile([128, APM], BF16, tag="rm0")
            rm1 = cpool.tile([72, APM], BF16, tag="rm1")

            CH = 8  # masks per chunk
            for ci, m0 in enumerate(range(0, APM, CH)):
                mc = min(CH, APM - m0)
                wc = mc * FW
                j0 = m0 * FW
                t0 = pool.tile([128, wc], BF16, tag="t0")
                t1 = pool.tile([72, wc], BF16, tag="t1")
                # gpsimd DMA casts fp32 -> bf16 in flight; sign/positivity
                # is preserved for all magnitudes this input can contain.
                nc.gpsimd.dma_start(
                    out=t0[:].rearrange("p (m c) -> p m c", c=FW),
                    in_=segs_r[0:128, m0 : m0 + mc],
                )
                nc.gpsimd.dma_start(
                    out=t1[:].rearrange("p (m c) -> p m c", c=FW),
                    in_=segs_r[128:FH, m0 : m0 + mc],
                )
                nc.vector.tensor_reduce(
                    out=rm0[:, m0 : m0 + mc],
                    in_=t0[:].rearrange("p (m c) -> p m c", c=FW),
                    axis=mybir.AxisListType.X,
                    op=mybir.AluOpType.max,
                )
                nc.vector.tensor_reduce(
                    out=rm1[:, m0 : m0 + mc],
                    in_=t1[:].rearrange("p (m c) -> p m c", c=FW),
                    axis=mybir.AxisListType.X,
                    op=mybir.AluOpType.max,
                )
                r0 = pool.tile([128, wc], BF16, tag="r0")
                r1 = pool.tile([72, wc], BF16, tag="r1")
                nc.scalar.activation(r0[:], t0[:], mybir.ActivationFunctionType.Relu)
                nc.scalar.activation(r1[:], t1[:], mybir.ActivationFunctionType.Relu)

                subs = [(c0, min(512, wc - c0)) for c0 in range(0, wc, 512)]
                full = [s for s in subs if s[1] == 512]
                ragged = [s for s in subs if s[1] < 512]
                for g0 in range(0, len(full), 3):
                    group = full[g0 : g0 + 3]
                    # chunk sums as 33-row replicated bands at bases 0/32/64
                    # of one bank; one engine copy then moves all of them
                    pg = psum.tile([96, 512], F32, tag="ps")
                    top = 32 * len(group)
                    for li, (c0, n512) in enumerate(group):
                        base = 32 * li
                        nc.tensor.matmul(
                            pg[base : base + 32, :],
                            ones[:, 0:32],
                            r0[:, c0 : c0 + 512],
                            start=True,
                            stop=False,
                        )
                        nc.tensor.matmul(
                            pg[base : base + 32, :],
                            ones[0:72, 0:32],
                            r1[:, c0 : c0 + 512],
                            start=False,
                            stop=True,
                        )
                    cs = pool.tile([96, 512], F32, tag="cs")
                    if (ci + g0) % 2 == 0:
                        nc.scalar.copy(out=cs[:top, :], in_=pg[:top, :])
                    else:
                        nc.vector.tensor_copy(out=cs[:top, :], in_=pg[:top, :])
                    for li, (c0, n512) in enumerate(group):
                        nc.sync.dma_start(
                            out=colsum[:, j0 + c0 : j0 + c0 + n512],
                            in_=cs[32 * li : 32 * li + 1, :n512],
                        )
                for c0, n512 in ragged:
                    ps = psum.tile([1, 512], F32, tag="psr")
                    nc.tensor.matmul(
                        ps[:, :n512],
                        ones[:, 0:1],
                        r0[:, c0 : c0 + n512],
                        start=True,
                        stop=False,
                    )
                    nc.tensor.matmul(
                        ps[:, :n512],
                        ones[0:72, 0:1],
                        r1[:, c0 : c0 + n512],
                        start=False,
                        stop=True,
                    )
                    csr = pool.tile([1, 512], F32, tag="csr")
                    nc.vector.tensor_copy(out=csr[:, :n512], in_=ps[:, :n512])
                    nc.sync.dma_start(
                        out=colsum[:, j0 + c0 : j0 + c0 + n512], in_=csr[:, :n512]
                    )
            nc.sync.dma_start(out=rowmax[0:128, :], in_=rm0[:])
            nc.sync.dma_start(out=rowmax[128:FH, :], in_=rm1[:])
    if split:
        _split_multi_waits(nc)
    return nc


# ----------------------------------------------------------------- pass B

def _resize_mat():
    """Exact jax bilinear 200->1024 operator H [200, 1024] (fp32)."""
    import jax
    import jax.numpy as jnp

    cpu = jax.devices("cpu")[0]
    with jax.default_device(cpu):
        eye = jnp.eye(FH, dtype=jnp.float32)
        h = jax.image.resize(eye, (FH, IW), method="bilinear")
        h = np.asarray(h, dtype=np.float32)
    # block-support structure the kernel relies on
    assert not h[128:, 0:512].any(), "H left-half support exceeds rows 0:128"
    assert not h[0:96, 512:].any(), "H right-half support exceeds rows 96:200"
    return h


def _build_pass_b(split=True):
    nc = bass.Bass()
    segs = nc.dram_tensor("segs", [BPM, FH, FW], F32, kind="ExternalInput")
    wl = nc.dram_tensor("wl", [128, 512], F32, kind="ExternalInput")
    wr = nc.dram_tensor("wr", [104, 512], F32, kind="ExternalInput")
    masks = nc.dram_tensor("masks", [BPM, IH, IW], U8, kind="ExternalOutput")

    with TileContext(nc) as tc:
        with (
            tc.tile_pool(name="const", bufs=1) as cpool,
            tc.tile_pool(name="sbuf", bufs=3) as pool,
            tc.tile_pool(name="ut", bufs=2) as utpool,
            tc.tile_pool(name="ou", bufs=4) as opool,
            tc.tile_pool(name="psA", bufs=1, space="PSUM") as psA,
            tc.tile_pool(name="psB", bufs=1, space="PSUM") as psB,
            tc.tile_pool(name="ps2", bufs=2, space="PSUM") as ps2pool,
        ):
            wlt = cpool.tile([128, 512], F32, tag="wl")
            wrt = cpool.tile([104, 512], F32, tag="wr")
            nc.sync.dma_start(out=wlt[:], in_=wl[:])
            nc.sync.dma_start(out=wrt[:], in_=wr[:])

            for m in range(BPM):
                ina = pool.tile([128, FW], F32, tag="ina")
                inb = pool.tile([104, FW], F32, tag="inb")
                nc.sync.dma_start(out=ina[:], in_=segs[m, 0:128, :])
                nc.sync.dma_start(out=inb[:], in_=segs[m, 96:FH, :])

                # stage 1 (vertical): UT_X[c, R] = sum_r in[r, c] * H[r, R]
                uta = psA.tile([128, IH], F32, tag="uta")
                utb = psB.tile([104, IH], F32, tag="utb")
                nc.tensor.matmul(
                    uta[:, 0:512], ina[:, 0:128], wlt[:], start=True, stop=True
                )
                nc.tensor.matmul(
                    uta[:, 512:1024], inb[:, 0:128], wrt[:], start=True, stop=True
                )
                nc.tensor.matmul(
                    utb[:, 0:512], ina[:, 96:FW], wlt[:], start=True, stop=True
                )
                nc.tensor.matmul(
                    utb[:, 512:1024], inb[:, 96:FW], wrt[:], start=True, stop=True
                )

                ua = utpool.tile([128, IH], F32, tag="ua")
                ub = utpool.tile([104, IH], F32, tag="ub")
                nc.scalar.copy(out=ua[:], in_=uta[:])
                nc.vector.tensor_copy(out=ub[:], in_=utb[:])

                # stage 2 (horizontal) + threshold + store, per 128-row block
                for a in range(8):
                    ps2 = ps2pool.tile([128, IW], F32, tag="ps2")
                    nc.tensor.matmul(
                        ps2[:, 0:512],
                        ua[:, 128 * a : 128 * (a + 1)],
                        wlt[:],
                        start=True,
                        stop=True,
                    )
                    nc.tensor.matmul(
                        ps2[:, 512:1024],
                        ub[:, 128 * a : 128 * (a + 1)],
                        wrt[:],
                        start=True,
                        stop=True,
                    )
                    ou = opool.tile([128, IW], U8, tag="ou")
                    if a % 2 == 0:
                        nc.scalar.sign(ou[:], ps2[:])
                    else:
                        nc.vector.tensor_scalar(
                            ou[:], ps2[:], 0.0, None, op0=mybir.AluOpType.is_gt
                        )
                    nc.sync.dma_start(
                        out=masks[m, 128 * a : 128 * (a + 1), :], in_=ou[:]
                    )
    if split:
        _split_multi_waits(nc)
    return nc


# ----------------------------------------------------------------- host glue

def _nms_glue(cls_logits, boxes, non_empty):
    """Mirror of the reference scoring/NMS math on jax CPU."""
    import jax
    import jax.numpy as jnp

    cpu = jax.devices("cpu")[0]
    with jax.default_device(cpu):
        cls_j = jnp.asarray(cls_logits, dtype=jnp.float32)
        boxes_j = jnp.asarray(boxes, dtype=jnp.float32)
        ne_j = jnp.asarray(non_empty)

        scores = jax.nn.sigmoid(cls_j[:, :-1])
        scores = jnp.where(ne_j[:, None], scores, -1.0).reshape(-1)
        feat_ids = jnp.repeat(jnp.arange(N), C)
        labels = jnp.tile(jnp.arange(C), N)

        k = min(NMS_CAND, N * C)
        cand_scores, cand_ids = jax.lax.top_k(scores, k)
        cand_feats = feat_ids[cand_ids]
        cand_labels = labels[cand_ids]
        cand_boxes = boxes_j[cand_feats]

        offset = cand_labels.astype(jnp.float32)[:, None] * (float(max(FH, FW)) + 1.0)
        b = cand_boxes + offset
        x1, y1, x2, y2 = b[:, 0], b[:, 1], b[:, 2], b[:, 3]
        area = (x2 - x1) * (y2 - y1)
        ix1 = jnp.maximum(x1[:, None], x1[None, :])
        iy1 = jnp.maximum(y1[:, None], y1[None, :])
        ix2 = jnp.minimum(x2[:, None], x2[None, :])
        iy2 = jnp.minimum(y2[:, None], y2[None, :])
        inter = jnp.clip(ix2 - ix1, 0.0) * jnp.clip(iy2 - iy1, 0.0)
        union = area[:, None] + area[None, :] - inter
        iou = inter / jnp.maximum(union, 1e-9)

        iou_np = np.asarray(iou)
        gt = iou_np > NMS_THR
        idx = np.arange(k)
        keep = np.ones((k,), dtype=bool)
        for i in range(k):
            if keep[i]:
                keep &= ~(gt[i] & (idx > i))
        keep = keep & np.asarray(cand_scores >= 0.0)

        kept_scores = jnp.where(jnp.asarray(keep), cand_scores, -jnp.inf)
        sel_scores, sel = jax.lax.top_k(kept_scores, MAX_SEGS)
        sel_feats = cand_feats[sel]
        sel_labels = cand_labels[sel]

    return (
        np.asarray(sel_feats),
        np.asarray(sel_labels, dtype=np.int32),
        np.asarray(sel_scores, dtype=np.float32),
    )


# ----------------------------------------------------------------- driver

_cache = {}


def _get(name, builder):
    if name not in _cache:
        _cache[name] = builder()
    return _cache[name]


def _run_spmd(nc, in_maps):
    """run_bass_kernel_spmd with retries for transient device wedges."""
    last = None
    for attempt in range(3):
        try:
            return run_bass_kernel_spmd(nc, in_maps, core_ids=list(range(NCORES)))
        except Exception as e:  # noqa: BLE001 - NRT wedges surface as RuntimeError
            last = e
            import time

            time.sleep(2.0 * (attempt + 1))
    raise last


def kernel(cls_logits, seg_logits):
    cls_logits = np.ascontiguousarray(np.asarray(cls_logits, dtype=np.float32))
    seg_logits = np.ascontiguousarray(np.asarray(seg_logits, dtype=np.float32))
    assert cls_logits.shape == (N, NCLS) and seg_logits.shape == (N, FH, FW)

    # ---- pass A: per-mask occupancy stats
    nc_a = _get("nc_a", _build_pass_a)
    seg_pad = np.zeros((NCORES * APM, FH, FW), dtype=np.float32)
    seg_pad[:N] = seg_logits
    in_maps = [
        {"segs": np.ascontiguousarray(seg_pad[c * APM : (c + 1) * APM])}
        for c in range(NCORES)
    ]
    res_a = _run_spmd(nc_a, in_maps)

    rowmax = np.concatenate(
        [np.asarray(res_a.results[c]["rowmax"], dtype=np.float32).T for c in range(NCORES)],
        axis=0,
    )[:N]  # [N, FH]
    colsum = np.concatenate(
        [res_a.results[c]["colsum"].reshape(APM, FW) for c in range(NCORES)], axis=0
    )[:N]  # [N, FW]

    row_any = rowmax > 0.0
    col_any = colsum > 0.0
    x1 = np.argmax(col_any, axis=1)
    x2 = FW - np.argmax(col_any[:, ::-1], axis=1)
    y1 = np.argmax(row_any, axis=1)
    y2 = FH - np.argmax(row_any[:, ::-1], axis=1)
    boxes = np.stack([x1, y1, x2, y2], axis=-1).astype(np.float32)
    non_empty = col_any.any(axis=1)

    # ---- host NMS glue (exact mirror of reference math)
    sel_feats, sel_labels, sel_scores, = _nms_glue(cls_logits, boxes, non_empty)

    # ---- pass B: upsample selected masks
    nc_b = _get("nc_b", _build_pass_b)
    h = _get("h", _resize_mat)
    w_l = np.ascontiguousarray(h[0:128, 0:512])
    w_r = np.ascontiguousarray(h[96:200, 512:1024])

    feats_pad = np.zeros((NCORES * BPM,), dtype=np.int64)
    feats_pad[:MAX_SEGS] = sel_feats
    gathered = seg_logits[feats_pad]  # [104, FH, FW]
    in_maps_b = [
        {
            "segs": np.ascontiguousarray(gathered[c * BPM : (c + 1) * BPM]),
            "wl": w_l,
            "wr": w_r,
        }
        for c in range(NCORES)
    ]
    res_b = _run_spmd(nc_b, in_maps_b)

    out_masks = np.concatenate(
        [res_b.results[c]["masks"] for c in range(NCORES)], axis=0
    )[:MAX_SEGS].view(np.bool_)

    batch_ids = np.zeros((MAX_SEGS,), dtype=np.int32)
    return sel_labels, out_masks, sel_scores, batch_ids


# revision 18
# speedup vs baseline: 1.0058x; 1.0058x over previous
"""Trainium2 kernel for the BaseSegHead NMS-detection problem.

Pipeline (full inputs in, full outputs out):
  pass A (device, data-parallel over masks): per-mask row/col occupancy
      stats of (seg_logits > 0) -> host turns them into boxes/non_empty.
  host glue (jax CPU, mirrors the reference math op-for-op): sigmoid
      scores, top-k candidates, class-aware greedy NMS, top max_segs.
  pass B (device, data-parallel over selected masks): bilinear 200->1024
      upsample as two chained PE matmuls per mask + >0 threshold to u8.
"""

import sys

sys.path.insert(0, "/opt/trn_rl_repo")

import numpy as np

import concourse.bass as bass
import concourse.mybir as mybir
from concourse.bass_utils import run_bass_kernel_spmd
from concourse.tile import TileContext

# problem constants (hardcoded per spec)
N, NCLS = 300, 81
C = NCLS - 1
FH = FW = 200
IH = IW = 1024
NMS_CAND = 1000
NMS_THR = 0.65
MAX_SEGS = 100
NCORES = 8

APM = 38   # masks per core, pass A (8*38 = 304 >= 300)
BPM = 13   # masks per core, pass B (8*13 = 104 >= 100)

F32 = mybir.dt.float32
BF16 = mybir.dt.bfloat16
U8 = mybir.dt.uint8


_SW_CTR = [0]


def _split_multi_waits(nc):
    """Move extra sem waits onto same-engine NoOps before each instruction.

    This walrus build accepts at most one sync wait per instruction
    (CTRL, S3_LW, ... encodings); Tile emits one wait per pending proc.
    """
    for f in nc.m.functions:
        for b in f.blocks:
            out = []
            changed = False
            for inst in b.instructions:
                si = inst.sync_info
                waits = list(si.on_wait) if si is not None else []
                if len(waits) > 1:
                    for w in waits[:-1]:
                        _SW_CTR[0] += 1
                        nop = mybir.InstNoOp(name=f"I-sw{_SW_CTR[0]}", ins=[], outs=[])
                        nop.engine = inst.engine
                        nop.sync_info = mybir.SyncInfo(on_wait=[w], on_update=[])
                        out.append(nop)
                    inst.sync_info = mybir.SyncInfo(
                        on_wait=[waits[-1]], on_update=list(si.on_update)
                    )
                    changed = True
                out.append(inst)
            if changed:
                b.instructions = out
                assert len(b.instructions) == len(out)


# ----------------------------------------------------------------- pass A

def _build_pass_a(split=True):
    nc = bass.Bass()
    segs = nc.dram_tensor("segs", [APM, FH, FW], F32, kind="ExternalInput")
    rowmax = nc.dram_tensor("rowmax", [FH, APM], BF16, kind="ExternalOutput")
    colsum = nc.dram_tensor("colsum", [1, APM * FW], F32, kind="ExternalOutput")

    W = APM * FW  # 7600
    segs_r = segs.rearrange("m r c -> r m c")

    with TileContext(nc) as tc:
        with (
            tc.tile_pool(name="const", bufs=1) as cpool,
            tc.tile_pool(name="sbuf", bufs=3) as pool,
            tc.tile_pool(name="psum", bufs=4, space="PSUM") as psum,
        ):
            ones = cpool.tile([128, 32], BF16, tag="ones")
            nc.vector.memset(ones[:], 1.0)
            rm0 = cpool.tile([128, APM], BF16, tag="rm0")
            rm1 = cpool.tile([72, APM], BF16, tag="rm1")

            CH = 8  # masks per chunk
            for ci, m0 in enumerate(range(0, APM, CH)):
                mc = min(CH, APM - m0)
                wc = mc * FW
                j0 = m0 * FW
                t0 = pool.tile([128, wc], BF16, tag="t0")
                t1 = pool.tile([72, wc], BF16, tag="t1")
                # gpsimd DMA casts fp32 -> bf16 in flight; sign/positivity
                # is preserved for all magnitudes this input can contain.
                nc.gpsimd.dma_start(
                    out=t0[:].rearrange("p (m c) -> p m c", c=FW),
                    in_=segs_r[0:128, m0 : m0 + mc],
                )
                nc.gpsimd.dma_start(
                    out=t1[:].rearrange("p (m c) -> p m c", c=FW),
                    in_=segs_r[128:FH, m0 : m0 + mc],
                )
                nc.vector.tensor_reduce(
                    out=rm0[:, m0 : m0 + mc],
                    in_=t0[:].rearrange("p (m c) -> p m c", c=FW),
                    axis=mybir.AxisListType.X,
                    op=mybir.AluOpType.max,
                )
                nc.vector.tensor_reduce(
                    out=rm1[:, m0 : m0 + mc],
                    in_=t1[:].rearrange("p (m c) -> p m c", c=FW),
                    axis=mybir.AxisListType.X,
                    op=mybir.AluOpType.max,
                )
                r0 = pool.tile([128, wc], BF16, tag="r0")
                r1 = pool.tile([72, wc], BF16, tag="r1")
                nc.scalar.activation(r0[:], t0[:], mybir.ActivationFunctionType.Relu)
                nc.scalar.activation(r1[:], t1[:], mybir.ActivationFunctionType.Relu)

                subs = [(c0, min(512, wc - c0)) for c0 in range(0, wc, 512)]
                full = [s for s in subs if s[1] == 512]
                ragged = [s for s in subs if s[1] < 512]
                for g0 in range(0, len(full), 3):
                    group = full[g0 : g0 + 3]
                    # chunk sums as 33-row replicated bands at bases 0/32/64
                    # of one bank; one engine copy then moves all of them
                    pg = psum.tile([96, 512], F32, tag="ps")
                    top = 32 * len(group)
                    for li, (c0, n512) in enumerate(group):
                        base = 32 * li
                        nc.tensor.matmul(
                            pg[base : base + 32, :],
                            ones[:, 0:32],
                            r0[:, c0 : c0 + 512],
                            start=True,
                            stop=False,
                        )
                        nc.tensor.matmul(
                            pg[base : base + 32, :],
                            ones[0:72, 0:32],
                            r1[:, c0 : c0 + 512],
                            start=False,
                            stop=True,
                        )
                    cs = pool.tile([96, 512], F32, tag="cs")
                    if (ci + g0) % 2 == 0:
                        nc.scalar.copy(out=cs[:top, :], in_=pg[:top, :])
                    else:
                        nc.vector.tensor_copy(out=cs[:top, :], in_=pg[:top, :])
                    for li, (c0, n512) in enumerate(group):
                        nc.sync.dma_start(
                            out=colsum[:, j0 + c0 : j0 + c0 + n512],
                            in_=cs[32 * li : 32 * li + 1, :n512],
                        )
                for c0, n512 in ragged:
                    ps = psum.tile([1, 512], F32, tag="psr")
                    nc.tensor.matmul(
                        ps[:, :n512],
                        ones[:, 0:1],
                        r0[:, c0 : c0 + n512],
                        start=True,
                        stop=False,
                    )
                    nc.tensor.matmul(
                        ps[:, :n512],
                        ones[0:72, 0:1],
                        r1[:, c0 : c0 + n512],
                        start=False,
                        stop=True,
                    )
                    csr = pool.tile([1, 512], F32, tag="csr")
                    nc.vector.tensor_copy(out=csr[:, :n512], in_=ps[:, :n512])
                    nc.sync.dma_start(
                        out=colsum[:, j0 + c0 : j0 + c0 + n512], in_=csr[:, :n512]
                    )
            nc.sync.dma_start(out=rowmax[0:128, :], in_=rm0[:])
            nc.sync.dma_start(out=rowmax[128:FH, :], in_=rm1[:])
    if split:
        _split_multi_waits(nc)
    return nc


# ----------------------------------------------------------------- pass B

def _resize_mat():
    """Exact jax bilinear 200->1024 operator H [200, 1024] (fp32)."""
    import jax
    import jax.numpy as jnp

    cpu = jax.devices("cpu")[0]
    with jax.default_device(cpu):
        eye = jnp.eye(FH, dtype=jnp.float32)
        h = jax.image.resize(eye, (FH, IW), method="bilinear")
        h = np.asarray(h, dtype=np.float32)
    # block-support structure the kernel relies on
    assert not h[128:, 0:512].any(), "H left-half support exceeds rows 0:128"
    assert not h[0:96, 512:].any(), "H right-half support exceeds rows 96:200"
    return h


def _build_pass_b(split=True):
    nc = bass.Bass()
    segs = nc.dram_tensor("segs", [BPM, FH, FW], F32, kind="ExternalInput")
    wl = nc.dram_tensor("wl", [128, 512], F32, kind="ExternalInput")
    wr = nc.dram_tensor("wr", [104, 512], F32, kind="ExternalInput")
    masks = nc.dram_tensor("masks", [BPM, IH, IW], U8, kind="ExternalOutput")

    with TileContext(nc) as tc:
        with (
            tc.tile_pool(name="const", bufs=1) as cpool,
            tc.tile_pool(name="sbuf", bufs=4) as pool,
            tc.tile_pool(name="ut", bufs=3) as utpool,
            tc.tile_pool(name="ou", bufs=6) as opool,
            tc.tile_pool(name="psA", bufs=1, space="PSUM") as psA,
            tc.tile_pool(name="psB", bufs=1, space="PSUM") as psB,
            tc.tile_pool(name="ps2", bufs=2, space="PSUM") as ps2pool,
        ):
            wlt = cpool.tile([128, 512], F32, tag="wl")
            wrt = cpool.tile([104, 512], F32, tag="wr")
            nc.sync.dma_start(out=wlt[:], in_=wl[:])
            nc.sync.dma_start(out=wrt[:], in_=wr[:])

            for m in range(BPM):
                ina = pool.tile([128, FW], F32, tag="ina")
                inb = pool.tile([104, FW], F32, tag="inb")
                nc.sync.dma_start(out=ina[:], in_=segs[m, 0:128, :])
                nc.sync.dma_start(out=inb[:], in_=segs[m, 96:FH, :])

                # stage 1 (vertical): UT_X[c, R] = sum_r in[r, c] * H[r, R]
                uta = psA.tile([128, IH], F32, tag="uta")
                utb = psB.tile([104, IH], F32, tag="utb")
                nc.tensor.matmul(
                    uta[:, 0:512], ina[:, 0:128], wlt[:], start=True, stop=True
                )
                nc.tensor.matmul(
                    uta[:, 512:1024], inb[:, 0:128], wrt[:], start=True, stop=True
                )
                nc.tensor.matmul(
                    utb[:, 0:512], ina[:, 96:FW], wlt[:], start=True, stop=True
                )
                nc.tensor.matmul(
                    utb[:, 512:1024], inb[:, 96:FW], wrt[:], start=True, stop=True
                )

                ua = utpool.tile([128, IH], F32, tag="ua")
                ub = utpool.tile([104, IH], F32, tag="ub")
                nc.scalar.copy(out=ua[:], in_=uta[:])
                nc.vector.tensor_copy(out=ub[:], in_=utb[:])

                # stage 2 (horizontal) + threshold + store, per 128-row block
                for a in range(8):
                    ps2 = ps2pool.tile([128, IW], F32, tag="ps2")
                    nc.tensor.matmul(
                        ps2[:, 0:512],
                        ua[:, 128 * a : 128 * (a + 1)],
                        wlt[:],
                        start=True,
                        stop=True,
                    )
                    nc.tensor.matmul(
                        ps2[:, 512:1024],
                        ub[:, 128 * a : 128 * (a + 1)],
                        wrt[:],
                        start=True,
                        stop=True,
                    )
                    ou = opool.tile([128, IW], U8, tag="ou")
                    if a % 2 == 0:
                        nc.scalar.sign(ou[:], ps2[:])
                    else:
                        nc.vector.tensor_scalar(
                            ou[:], ps2[:], 0.0, None, op0=mybir.AluOpType.is_gt
                        )
                    nc.sync.dma_start(
                        out=masks[m, 128 * a : 128 * (a + 1), :], in_=ou[:]
                    )
    if split:
        _split_multi_waits(nc)
    return nc


# ----------------------------------------------------------------- host glue

def _nms_glue(cls_logits, boxes, non_empty):
    """Mirror of the reference scoring/NMS math on jax CPU."""
    import jax
    import jax.numpy as jnp

    cpu = jax.devices("cpu")[0]
    with jax.default_device(cpu):
        cls_j = jnp.asarray(cls_logits, dtype=jnp.float32)
        boxes_j = jnp.asarray(boxes, dtype=jnp.float32)
        ne_j = jnp.asarray(non_empty)

        scores = jax.nn.sigmoid(cls_j[:, :-1])
        scores = jnp.where(ne_j[:, None], scores, -1.0).reshape(-1)
        feat_ids = jnp.repeat(jnp.arange(N), C)
        labels = jnp.tile(jnp.arange(C), N)

        k = min(NMS_CAND, N * C)
        cand_scores, cand_ids = jax.lax.top_k(scores, k)
        cand_feats = feat_ids[cand_ids]
        cand_labels = labels[cand_ids]
        cand_boxes = boxes_j[cand_feats]

        offset = cand_labels.astype(jnp.float32)[:, None] * (float(max(FH, FW)) + 1.0)
        b = cand_boxes + offset
        x1, y1, x2, y2 = b[:, 0], b[:, 1], b[:, 2], b[:, 3]
        area = (x2 - x1) * (y2 - y1)
        ix1 = jnp.maximum(x1[:, None], x1[None, :])
        iy1 = jnp.maximum(y1[:, None], y1[None, :])
        ix2 = jnp.minimum(x2[:, None], x2[None, :])
        iy2 = jnp.minimum(y2[:, None], y2[None, :])
        inter = jnp.clip(ix2 - ix1, 0.0) * jnp.clip(iy2 - iy1, 0.0)
        union = area[:, None] + area[None, :] - inter
        iou = inter / jnp.maximum(union, 1e-9)

        iou_np = np.asarray(iou)
        gt = iou_np > NMS_THR
        idx = np.arange(k)
        keep = np.ones((k,), dtype=bool)
        for i in range(k):
            if keep[i]:
                keep &= ~(gt[i] & (idx > i))
        keep = keep & np.asarray(cand_scores >= 0.0)

        kept_scores = jnp.where(jnp.asarray(keep), cand_scores, -jnp.inf)
        sel_scores, sel = jax.lax.top_k(kept_scores, MAX_SEGS)
        sel_feats = cand_feats[sel]
        sel_labels = cand_labels[sel]

    return (
        np.asarray(sel_feats),
        np.asarray(sel_labels, dtype=np.int32),
        np.asarray(sel_scores, dtype=np.float32),
    )


# ----------------------------------------------------------------- driver

_cache = {}


def _get(name, builder):
    if name not in _cache:
        _cache[name] = builder()
    return _cache[name]


def _run_spmd(nc, in_maps):
    """run_bass_kernel_spmd with retries for transient device wedges."""
    last = None
    for attempt in range(3):
        try:
            return run_bass_kernel_spmd(nc, in_maps, core_ids=list(range(NCORES)))
        except Exception as e:  # noqa: BLE001 - NRT wedges surface as RuntimeError
            last = e
            import time

            time.sleep(2.0 * (attempt + 1))
    raise last


def kernel(cls_logits, seg_logits):
    cls_logits = np.ascontiguousarray(np.asarray(cls_logits, dtype=np.float32))
    seg_logits = np.ascontiguousarray(np.asarray(seg_logits, dtype=np.float32))
    assert cls_logits.shape == (N, NCLS) and seg_logits.shape == (N, FH, FW)

    # ---- pass A: per-mask occupancy stats
    nc_a = _get("nc_a", _build_pass_a)
    seg_pad = np.zeros((NCORES * APM, FH, FW), dtype=np.float32)
    seg_pad[:N] = seg_logits
    in_maps = [
        {"segs": np.ascontiguousarray(seg_pad[c * APM : (c + 1) * APM])}
        for c in range(NCORES)
    ]
    res_a = _run_spmd(nc_a, in_maps)

    rowmax = np.concatenate(
        [np.asarray(res_a.results[c]["rowmax"], dtype=np.float32).T for c in range(NCORES)],
        axis=0,
    )[:N]  # [N, FH]
    colsum = np.concatenate(
        [res_a.results[c]["colsum"].reshape(APM, FW) for c in range(NCORES)], axis=0
    )[:N]  # [N, FW]

    row_any = rowmax > 0.0
    col_any = colsum > 0.0
    x1 = np.argmax(col_any, axis=1)
    x2 = FW - np.argmax(col_any[:, ::-1], axis=1)
    y1 = np.argmax(row_any, axis=1)
    y2 = FH - np.argmax(row_any[:, ::-1], axis=1)
    boxes = np.stack([x1, y1, x2, y2], axis=-1).astype(np.float32)
    non_empty = col_any.any(axis=1)

    # ---- host NMS glue (exact mirror of reference math)
    sel_feats, sel_labels, sel_scores, = _nms_glue(cls_logits, boxes, non_empty)

    # ---- pass B: upsample selected masks
    nc_b = _get("nc_b", _build_pass_b)
    h = _get("h", _resize_mat)
    w_l = np.ascontiguousarray(h[0:128, 0:512])
    w_r = np.ascontiguousarray(h[96:200, 512:1024])

    feats_pad = np.zeros((NCORES * BPM,), dtype=np.int64)
    feats_pad[:MAX_SEGS] = sel_feats
    gathered = seg_logits[feats_pad]  # [104, FH, FW]
    in_maps_b = [
        {
            "segs": np.ascontiguousarray(gathered[c * BPM : (c + 1) * BPM]),
            "wl": w_l,
            "wr": w_r,
        }
        for c in range(NCORES)
    ]
    res_b = _run_spmd(nc_b, in_maps_b)

    out_masks = np.concatenate(
        [res_b.results[c]["masks"] for c in range(NCORES)], axis=0
    )[:MAX_SEGS].view(np.bool_)

    batch_ids = np.zeros((MAX_SEGS,), dtype=np.int32)
    return sel_labels, out_masks, sel_scores, batch_ids


# revision 19
# speedup vs baseline: 1.1439x; 1.1373x over previous
"""Trainium2 kernel for the BaseSegHead NMS-detection problem.

Pipeline (full inputs in, full outputs out):
  pass A (device, data-parallel over masks): per-mask row/col occupancy
      stats of (seg_logits > 0) -> host turns them into boxes/non_empty.
  host glue (jax CPU, mirrors the reference math op-for-op): sigmoid
      scores, top-k candidates, class-aware greedy NMS, top max_segs.
  pass B (device, data-parallel over selected masks): bilinear 200->1024
      upsample as two chained PE matmuls per mask + >0 threshold to u8.
"""

import sys

sys.path.insert(0, "/opt/trn_rl_repo")

import numpy as np

import concourse.bass as bass
import concourse.mybir as mybir
from concourse.bass_utils import run_bass_kernel_spmd
from concourse.tile import TileContext

# problem constants (hardcoded per spec)
N, NCLS = 300, 81
C = NCLS - 1
FH = FW = 200
IH = IW = 1024
NMS_CAND = 1000
NMS_THR = 0.65
MAX_SEGS = 100
NCORES = 8

APM = 38   # masks per core, pass A (8*38 = 304 >= 300)
BPM = 13   # masks per core, pass B (8*13 = 104 >= 100)

F32 = mybir.dt.float32
BF16 = mybir.dt.bfloat16
U8 = mybir.dt.uint8


_SW_CTR = [0]


def _split_multi_waits(nc):
    """Move extra sem waits onto same-engine NoOps before each instruction.

    This walrus build accepts at most one sync wait per instruction
    (CTRL, S3_LW, ... encodings); Tile emits one wait per pending proc.
    """
    for f in nc.m.functions:
        for b in f.blocks:
            out = []
            changed = False
            for inst in b.instructions:
                si = inst.sync_info
                waits = list(si.on_wait) if si is not None else []
                if len(waits) > 1:
                    for w in waits[:-1]:
                        _SW_CTR[0] += 1
                        nop = mybir.InstNoOp(name=f"I-sw{_SW_CTR[0]}", ins=[], outs=[])
                        nop.engine = inst.engine
                        nop.sync_info = mybir.SyncInfo(on_wait=[w], on_update=[])
                        out.append(nop)
                    inst.sync_info = mybir.SyncInfo(
                        on_wait=[waits[-1]], on_update=list(si.on_update)
                    )
                    changed = True
                out.append(inst)
            if changed:
                b.instructions = out
                assert len(b.instructions) == len(out)


# ----------------------------------------------------------------- pass A

def _build_pass_a(split=True):
    nc = bass.Bass()
    segs = nc.dram_tensor("segs", [APM, FH, FW], F32, kind="ExternalInput")
    rowmax = nc.dram_tensor("rowmax", [FH, APM], BF16, kind="ExternalOutput")
    colsum = nc.dram_tensor("colsum", [1, APM * FW], F32, kind="ExternalOutput")

    W = APM * FW  # 7600
    segs_r = segs.rearrange("m r c -> r m c")

    with TileContext(nc) as tc:
        with (
            tc.tile_pool(name="const", bufs=1) as cpool,
            tc.tile_pool(name="sbuf", bufs=3) as pool,
            tc.tile_pool(name="psum", bufs=4, space="PSUM") as psum,
        ):
            ones = cpool.tile([128, 32], BF16, tag="ones")
            nc.vector.memset(ones[:], 1.0)
            rm0 = cpool.tile([128, APM], BF16, tag="rm0")
            rm1 = cpool.tile([72, APM], BF16, tag="rm1")

            CH = 8  # masks per chunk
            for ci, m0 in enumerate(range(0, APM, CH)):
                mc = min(CH, APM - m0)
                wc = mc * FW
                j0 = m0 * FW
                t0 = pool.tile([128, wc], BF16, tag="t0")
                t1 = pool.tile([72, wc], BF16, tag="t1")
                # gpsimd DMA casts fp32 -> bf16 in flight; sign/positivity
                # is preserved for all magnitudes this input can contain.
                nc.gpsimd.dma_start(
                    out=t0[:].rearrange("p (m c) -> p m c", c=FW),
                    in_=segs_r[0:128, m0 : m0 + mc],
                )
                nc.gpsimd.dma_start(
                    out=t1[:].rearrange("p (m c) -> p m c", c=FW),
                    in_=segs_r[128:FH, m0 : m0 + mc],
                )
                nc.vector.tensor_reduce(
                    out=rm0[:, m0 : m0 + mc],
                    in_=t0[:].rearrange("p (m c) -> p m c", c=FW),
                    axis=mybir.AxisListType.X,
                    op=mybir.AluOpType.max,
                )
                nc.vector.tensor_reduce(
                    out=rm1[:, m0 : m0 + mc],
                    in_=t1[:].rearrange("p (m c) -> p m c", c=FW),
                    axis=mybir.AxisListType.X,
                    op=mybir.AluOpType.max,
                )
                r0 = pool.tile([128, wc], BF16, tag="r0")
                r1 = pool.tile([72, wc], BF16, tag="r1")
                nc.scalar.activation(r0[:], t0[:], mybir.ActivationFunctionType.Relu)
                nc.scalar.activation(r1[:], t1[:], mybir.ActivationFunctionType.Relu)

                subs = [(c0, min(512, wc - c0)) for c0 in range(0, wc, 512)]
                full = [s for s in subs if s[1] == 512]
                ragged = [s for s in subs if s[1] < 512]
                for g0 in range(0, len(full), 3):
                    group = full[g0 : g0 + 3]
                    # chunk sums as 33-row replicated bands at bases 0/32/64
                    # of one bank; one engine copy then moves all of them
                    pg = psum.tile([96, 512], F32, tag="ps")
                    top = 32 * len(group)
                    for li, (c0, n512) in enumerate(group):
                        base = 32 * li
                        nc.tensor.matmul(
                            pg[base : base + 32, :],
                            ones[:, 0:32],
                            r0[:, c0 : c0 + 512],
                            start=True,
                            stop=False,
                        )
                        nc.tensor.matmul(
                            pg[base : base + 32, :],
                            ones[0:72, 0:32],
                            r1[:, c0 : c0 + 512],
                            start=False,
                            stop=True,
                        )
                    cs = pool.tile([96, 512], F32, tag="cs")
                    if (ci + g0) % 2 == 0:
                        nc.scalar.copy(out=cs[:top, :], in_=pg[:top, :])
                    else:
                        nc.vector.tensor_copy(out=cs[:top, :], in_=pg[:top, :])
                    for li, (c0, n512) in enumerate(group):
                        nc.sync.dma_start(
                            out=colsum[:, j0 + c0 : j0 + c0 + n512],
                            in_=cs[32 * li : 32 * li + 1, :n512],
                        )
                for c0, n512 in ragged:
                    ps = psum.tile([1, 512], F32, tag="psr")
                    nc.tensor.matmul(
                        ps[:, :n512],
                        ones[:, 0:1],
                        r0[:, c0 : c0 + n512],
                        start=True,
                        stop=False,
                    )
                    nc.tensor.matmul(
                        ps[:, :n512],
                        ones[0:72, 0:1],
                        r1[:, c0 : c0 + n512],
                        start=False,
                        stop=True,
                    )
                    csr = pool.tile([1, 512], F32, tag="csr")
                    nc.vector.tensor_copy(out=csr[:, :n512], in_=ps[:, :n512])
                    nc.sync.dma_start(
                        out=colsum[:, j0 + c0 : j0 + c0 + n512], in_=csr[:, :n512]
                    )
            nc.sync.dma_start(out=rowmax[0:128, :], in_=rm0[:])
            nc.sync.dma_start(out=rowmax[128:FH, :], in_=rm1[:])
    if split:
        _split_multi_waits(nc)
    return nc


# ----------------------------------------------------------------- pass B

def _resize_mat():
    """Exact jax bilinear 200->1024 operator H [200, 1024] (fp32)."""
    import jax
    import jax.numpy as jnp

    cpu = jax.devices("cpu")[0]
    with jax.default_device(cpu):
        eye = jnp.eye(FH, dtype=jnp.float32)
        h = jax.image.resize(eye, (FH, IW), method="bilinear")
        h = np.asarray(h, dtype=np.float32)
    # block-support structure the kernel relies on
    assert not h[128:, 0:512].any(), "H left-half support exceeds rows 0:128"
    assert not h[0:96, 512:].any(), "H right-half support exceeds rows 96:200"
    return h


def _build_pass_b(split=True, bpm=BPM):
    nc = bass.Bass()
    segs = nc.dram_tensor("segs", [bpm, FH, FW], F32, kind="ExternalInput")
    wl = nc.dram_tensor("wl", [128, 512], F32, kind="ExternalInput")
    wr = nc.dram_tensor("wr", [104, 512], F32, kind="ExternalInput")
    masks = nc.dram_tensor("masks", [bpm, IH, IW], U8, kind="ExternalOutput")

    with TileContext(nc) as tc:
        with (
            tc.tile_pool(name="const", bufs=1) as cpool,
            tc.tile_pool(name="sbuf", bufs=4) as pool,
            tc.tile_pool(name="ut", bufs=3) as utpool,
            tc.tile_pool(name="ou", bufs=6) as opool,
            tc.tile_pool(name="psA", bufs=1, space="PSUM") as psA,
            tc.tile_pool(name="psB", bufs=1, space="PSUM") as psB,
            tc.tile_pool(name="ps2", bufs=2, space="PSUM") as ps2pool,
        ):
            wlt = cpool.tile([128, 512], F32, tag="wl")
            wrt = cpool.tile([104, 512], F32, tag="wr")
            nc.sync.dma_start(out=wlt[:], in_=wl[:])
            nc.sync.dma_start(out=wrt[:], in_=wr[:])

            for m in range(bpm):
                ina = pool.tile([128, FW], F32, tag="ina")
                inb = pool.tile([104, FW], F32, tag="inb")
                nc.sync.dma_start(out=ina[:], in_=segs[m, 0:128, :])
                nc.sync.dma_start(out=inb[:], in_=segs[m, 96:FH, :])

                # stage 1 (vertical): UT_X[c, R] = sum_r in[r, c] * H[r, R]
                uta = psA.tile([128, IH], F32, tag="uta")
                utb = psB.tile([104, IH], F32, tag="utb")
                nc.tensor.matmul(
                    uta[:, 0:512], ina[:, 0:128], wlt[:], start=True, stop=True
                )
                nc.tensor.matmul(
                    uta[:, 512:1024], inb[:, 0:128], wrt[:], start=True, stop=True
                )
                nc.tensor.matmul(
                    utb[:, 0:512], ina[:, 96:FW], wlt[:], start=True, stop=True
                )
                nc.tensor.matmul(
                    utb[:, 512:1024], inb[:, 96:FW], wrt[:], start=True, stop=True
                )

                ua = utpool.tile([128, IH], F32, tag="ua")
                ub = utpool.tile([104, IH], F32, tag="ub")
                nc.scalar.copy(out=ua[:], in_=uta[:])
                nc.vector.tensor_copy(out=ub[:], in_=utb[:])

                # stage 2 (horizontal) + threshold + store, per 128-row block
                for a in range(8):
                    ps2 = ps2pool.tile([128, IW], F32, tag="ps2")
                    nc.tensor.matmul(
                        ps2[:, 0:512],
                        ua[:, 128 * a : 128 * (a + 1)],
                        wlt[:],
                        start=True,
                        stop=True,
                    )
                    nc.tensor.matmul(
                        ps2[:, 512:1024],
                        ub[:, 128 * a : 128 * (a + 1)],
                        wrt[:],
                        start=True,
                        stop=True,
                    )
                    ou = opool.tile([128, IW], U8, tag="ou")
                    if a % 2 == 0:
                        nc.scalar.sign(ou[:], ps2[:])
                    else:
                        nc.vector.tensor_scalar(
                            ou[:], ps2[:], 0.0, None, op0=mybir.AluOpType.is_gt
                        )
                    nc.sync.dma_start(
                        out=masks[m, 128 * a : 128 * (a + 1), :], in_=ou[:]
                    )
    if split:
        _split_multi_waits(nc)
    return nc


# ----------------------------------------------------------------- host glue

def _nms_glue(cls_logits, boxes, non_empty):
    """Mirror of the reference scoring/NMS math on jax CPU."""
    import jax
    import jax.numpy as jnp

    cpu = jax.devices("cpu")[0]
    with jax.default_device(cpu):
        cls_j = jnp.asarray(cls_logits, dtype=jnp.float32)
        boxes_j = jnp.asarray(boxes, dtype=jnp.float32)
        ne_j = jnp.asarray(non_empty)

        scores = jax.nn.sigmoid(cls_j[:, :-1])
        scores = jnp.where(ne_j[:, None], scores, -1.0).reshape(-1)
        feat_ids = jnp.repeat(jnp.arange(N), C)
        labels = jnp.tile(jnp.arange(C), N)

        k = min(NMS_CAND, N * C)
        cand_scores, cand_ids = jax.lax.top_k(scores, k)
        cand_feats = feat_ids[cand_ids]
        cand_labels = labels[cand_ids]
        cand_boxes = boxes_j[cand_feats]

        offset = cand_labels.astype(jnp.float32)[:, None] * (float(max(FH, FW)) + 1.0)
        b = cand_boxes + offset
        x1, y1, x2, y2 = b[:, 0], b[:, 1], b[:, 2], b[:, 3]
        area = (x2 - x1) * (y2 - y1)
        ix1 = jnp.maximum(x1[:, None], x1[None, :])
        iy1 = jnp.maximum(y1[:, None], y1[None, :])
        ix2 = jnp.minimum(x2[:, None], x2[None, :])
        iy2 = jnp.minimum(y2[:, None], y2[None, :])
        inter = jnp.clip(ix2 - ix1, 0.0) * jnp.clip(iy2 - iy1, 0.0)
        union = area[:, None] + area[None, :] - inter
        iou = inter / jnp.maximum(union, 1e-9)

        iou_np = np.asarray(iou)
        gt = iou_np > NMS_THR
        idx = np.arange(k)
        keep = np.ones((k,), dtype=bool)
        for i in range(k):
            if keep[i]:
                keep &= ~(gt[i] & (idx > i))
        keep = keep & np.asarray(cand_scores >= 0.0)

        kept_scores = jnp.where(jnp.asarray(keep), cand_scores, -jnp.inf)
        sel_scores, sel = jax.lax.top_k(kept_scores, MAX_SEGS)
        sel_feats = cand_feats[sel]
        sel_labels = cand_labels[sel]

    return (
        np.asarray(sel_feats),
        np.asarray(sel_labels, dtype=np.int32),
        np.asarray(sel_scores, dtype=np.float32),
    )


# ----------------------------------------------------------------- driver

_cache = {}


def _get(name, builder):
    if name not in _cache:
        _cache[name] = builder()
    return _cache[name]


def _run_spmd(nc, in_maps):
    """run_bass_kernel_spmd with retries for transient device wedges."""
    last = None
    for attempt in range(3):
        try:
            return run_bass_kernel_spmd(nc, in_maps, core_ids=list(range(NCORES)))
        except Exception as e:  # noqa: BLE001 - NRT wedges surface as RuntimeError
            last = e
            import time

            time.sleep(2.0 * (attempt + 1))
    raise last


def kernel(cls_logits, seg_logits):
    cls_logits = np.ascontiguousarray(np.asarray(cls_logits, dtype=np.float32))
    seg_logits = np.ascontiguousarray(np.asarray(seg_logits, dtype=np.float32))
    assert cls_logits.shape == (N, NCLS) and seg_logits.shape == (N, FH, FW)

    # ---- pass A: per-mask occupancy stats
    nc_a = _get("nc_a", _build_pass_a)
    seg_pad = np.zeros((NCORES * APM, FH, FW), dtype=np.float32)
    seg_pad[:N] = seg_logits
    in_maps = [
        {"segs": np.ascontiguousarray(seg_pad[c * APM : (c + 1) * APM])}
        for c in range(NCORES)
    ]
    res_a = _run_spmd(nc_a, in_maps)

    rowmax = np.concatenate(
        [np.asarray(res_a.results[c]["rowmax"], dtype=np.float32).T for c in range(NCORES)],
        axis=0,
    )[:N]  # [N, FH]
    colsum = np.concatenate(
        [res_a.results[c]["colsum"].reshape(APM, FW) for c in range(NCORES)], axis=0
    )[:N]  # [N, FW]

    row_any = rowmax > 0.0
    col_any = colsum > 0.0
    x1 = np.argmax(col_any, axis=1)
    x2 = FW - np.argmax(col_any[:, ::-1], axis=1)
    y1 = np.argmax(row_any, axis=1)
    y2 = FH - np.argmax(row_any[:, ::-1], axis=1)
    boxes = np.stack([x1, y1, x2, y2], axis=-1).astype(np.float32)
    non_empty = col_any.any(axis=1)

    # ---- host NMS glue (exact mirror of reference math)
    sel_feats, sel_labels, sel_scores, = _nms_glue(cls_logits, boxes, non_empty)

    # ---- pass B: upsample each distinct selected mask once (class-aware
    # NMS can pick the same feat under several labels); replicate on host
    uniq, inv = np.unique(sel_feats, return_inverse=True)
    nuniq = len(uniq)
    bpm = max(1, -(-nuniq // NCORES))
    nc_b = _get(f"nc_b_{bpm}", lambda: _build_pass_b(bpm=bpm))
    h = _get("h", _resize_mat)
    w_l = np.ascontiguousarray(h[0:128, 0:512])
    w_r = np.ascontiguousarray(h[96:200, 512:1024])

    feats_pad = np.zeros((NCORES * bpm,), dtype=np.int64)
    feats_pad[:nuniq] = uniq
    gathered = seg_logits[feats_pad]
    in_maps_b = [
        {
            "segs": np.ascontiguousarray(gathered[c * bpm : (c + 1) * bpm]),
            "wl": w_l,
            "wr": w_r,
        }
        for c in range(NCORES)
    ]
    res_b = _run_spmd(nc_b, in_maps_b)

    uniq_masks = np.concatenate(
        [res_b.results[c]["masks"] for c in range(NCORES)], axis=0
    )[:nuniq]
    out_masks = np.ascontiguousarray(uniq_masks[inv]).view(np.bool_)

    batch_ids = np.zeros((MAX_SEGS,), dtype=np.int32)
    return sel_labels, out_masks, sel_scores, batch_ids
